# revision 1
# baseline (speedup 1.0000x reference)
"""CMSA (cross-modal self-attention) Trainium2 Bass kernel, v6.

Problem: two feature maps x,y of [B=4, C=256, H=64, W=64]. Per sample:
  q_y,k_y = 1x1conv(y) -> [32, N]; v_x = 1x1conv(x) -> [256, N]  (N=4096)
  att_y = softmax(q_y^T k_y); enhanced_x = v_x @ att_y^T + x
  (and symmetrically x->y). Output: (enhanced_x, enhanced_y).

Sharding: 8 independent attention problems = (4 samples) x (2 directions),
one per NeuronCore, SPMD. Per-core kernel computes one full attention.

Kernel math (per core):
  L^T[j,i] = sum_d k[d,j] q[d,i]     (k-tile stationary fp16)
  U^T[j,i] = exp(L^T[j,i])           (unnormalized bf16)
  T[i, 0:256] = sum_j U^T[j,i] V^T[j,c]   "transposed AV": U^T-slice is the
  T[i, 256]   = sum_j U^T[j,i]             stationary operand, [V^T | ones]
                                           (257 cols) is the moving operand;
                                           denominator rides as column 256
  out^T[i,c] = T[i,c] / T[i,256] + (feat_v^T[i,c] + bv[c])

v6 schedule (the exp stream on the Scalar engine is the bottleneck, so
everything is built to saturate it from ~12us on):
  - all weights arrive host-side pre-transposed/replicated; feat_qk is
    declared fp32r (bit-identical to fp32) so no on-device casts at all
  - projections are consumed per-512-chunk: as feature chunk nb lands, q/k
    project and block-0 QK pairs jp=2nb,2nb+1 issue immediately
  - QK pairs alternate PE row-sets {0,1}/{2,3} (q/k replicated 4x across
    partitions): the pair's two matmuls run concurrently and the next
    pair's weight loads overlap the in-flight pair
  - exp reads/writes flat 2D [128,1024] APs
  - V projection runs in bf16 from the host-staged bf16 feat_v; its psum
    drains are plain copies (bv is folded into the residual via a second
    host input (feat_v+bv) in bf16, which the DMA xbar transposes)
  - AV matmuls are interleaved 8-per-jp into the QK/exp stream with a
    one-group lag: block 1's first 4 jp slots run the V projection burst
    instead, giving the vTx drains headroom; the AV backlog (5 groups)
    drains in the tail
"""

import numpy as np

import concourse.bass as bass
import concourse.tile as tile
from concourse import bacc, mybir
from concourse.bass_utils import run_bass_kernel_spmd

C = 256
RD = 32
B = 4
N = 64 * 64  # 4096
NCORES = 8

IBLK = 512           # i-block size (query block)
NIB = N // IBLK      # 8
JT = 128             # j tile size
NJT = N // JT        # 32
ITPB = IBLK // 128   # 128-wide i-tiles per block = 4
VX = C + 1           # moving width of the AV matmul (values + ones column)

F32 = mybir.dt.float32
F32R = mybir.dt.float32r
BF16 = mybir.dt.bfloat16
F16 = mybir.dt.float16


def _build_bass():
    nc = bacc.Bacc(
        "TRN2",
        target_bir_lowering=False,
        debug=False,
        num_devices=NCORES,
    )

    feat_qk = nc.dram_tensor("feat_qk", [C, N], F32, kind="ExternalInput").ap()
    fv16 = nc.dram_tensor("fv16", [C, N], BF16, kind="ExternalInput").ap()
    fvb16 = nc.dram_tensor("fvb16", [C, N], BF16, kind="ExternalInput").ap()
    # host-packed weights: [cin_inner=128, cin_outer=2, cout]
    wqT = nc.dram_tensor("wqT", [128, 2, 4 * RD], F32R, kind="ExternalInput").ap()
    wkT = nc.dram_tensor("wkT", [128, 2, 4 * RD], F32R, kind="ExternalInput").ap()
    wvT = nc.dram_tensor("wvT", [128, 2, C], BF16, kind="ExternalInput").ap()
    bq4 = nc.dram_tensor("bq4", [4 * RD, 1], F32, kind="ExternalInput").ap()
    bk4 = nc.dram_tensor("bk4", [4 * RD, 1], F32, kind="ExternalInput").ap()
    # transposed output [i, c]; host flips back to [C, N]
    out = nc.dram_tensor("out_t", [N, C], F32, kind="ExternalOutput").ap()

    with tile.TileContext(nc) as tc:
        _kernel_body(nc, tc, feat_qk, fv16, fvb16, wqT, wkT, wvT, bq4, bk4, out)
    nc.compile()
    return nc


def _kernel_body(nc, tc, feat_qk, fv16, fvb16, wqT, wkT, wvT, bq4, bk4, out):
    Exp = mybir.ActivationFunctionType.Exp
    with (
        tc.tile_pool(name="singles", bufs=1) as singles,
        tc.tile_pool(name="work", bufs=4) as work,
        tc.tile_pool(name="opool", bufs=4) as opool,
        tc.tile_pool(name="upool", bufs=38) as upool,
        tc.tile_pool(name="qstage", bufs=6) as qstage,
        tc.tile_pool(name="qstager", bufs=3) as qstager,
        tc.tile_pool(name="vstage", bufs=8) as vstage,
        tc.tile_pool(name="qk_psum", bufs=3, space="PSUM") as qk_psum,
        tc.tile_pool(name="av_psum", bufs=2, space="PSUM") as av_psum,
    ):
        # ---- persistent SBUF ----
        # q/k: fp16, rows replicated 4x (row sets 0-31/32-63/64-95/96-127)
        q_sb = singles.tile([4 * RD, N], F16, tag="q")
        k_sb = singles.tile([4 * RD, N], F16, tag="k")
        # [V^T | ones] moving tiles: [j_inner, j_tile, VX] bf16 (col C = 1.0)
        vTx_sb = singles.tile([128, NJT, VX + 3], BF16, tag="vTx")
        # residual (feat_v + bv)^T tiles (bf16, from the DMA xbar transpose)
        fvT_sb = singles.tile([128, N // 128, C], BF16, tag="fvT")

        # warmup operands (gpsimd memsets keep the DVE free)
        wu_w = singles.tile([128, 128], BF16, tag="wu_w")
        wu_x = singles.tile([128, 512], BF16, tag="wu_x")
        dummy = singles.tile([128, 8], BF16, tag="dummy")
        nc.gpsimd.memset(wu_w, 1.0)
        nc.gpsimd.memset(wu_x, 1.0)
        # ones column of the AV moving operand
        nc.gpsimd.memset(vTx_sb[:, :, C : C + 1], 1.0)

        # ---- DMA issue ----
        # sync queue: the small q/k weight+bias transfers go first (the
        # projection chain stalls multi-us if they queue behind the feature
        # flood), then feat_qk chunks, feat_v chunks, the xbar transposes
        wqT_sb = singles.tile([128, 2, 4 * RD], F32R, tag="wqT")
        wkT_sb = singles.tile([128, 2, 4 * RD], F32R, tag="wkT")
        bq_sb = singles.tile([4 * RD, 1], F32, tag="bq")
        bk_sb = singles.tile([4 * RD, 1], F32, tag="bk")

        fqk_pn = feat_qk.rearrange("(a p) n -> p a n", a=2)
        fqk_chunks = []
        for nb in range(NIB):
            ns = bass.ts(nb, IBLK)
            fqk_st = qstage.tile([128, 2, IBLK], F32, tag="fqk_st")
            if nb == 0:
                # chunk 0 is the critical path: split per cin-half so the
                # first projection matmul overlaps the second half's DMA
                for co in range(2):
                    nc.sync.dma_start(
                        out=fqk_st[:, co, :],
                        in_=feat_qk[co * 128 : (co + 1) * 128, ns],
                    )
            else:
                nc.sync.dma_start(out=fqk_st, in_=fqk_pn[:, :, ns])
            fqk_chunks.append(fqk_st)
            if nb == 0:
                # weights slot in right after chunk 0 (the longest pole);
                # they are needed by the first projection at ~13us
                nc.sync.dma_start(out=bq_sb, in_=bq4)
                nc.sync.dma_start(out=bk_sb, in_=bk4)
                nc.sync.dma_start(out=wqT_sb, in_=wqT)
                nc.sync.dma_start(out=wkT_sb, in_=wkT)
        # feat_v chunks trail the fqk stream so fqk gets full bandwidth
        fv_pn = fv16.rearrange("(a p) n -> p a n", a=2)
        fv_chunks = []
        for nb in range(NIB):
            ns = bass.ts(nb, IBLK)
            fv_st = vstage.tile([128, 2, IBLK], BF16, tag="fv_st")
            nc.sync.dma_start(out=fv_st, in_=fv_pn[:, :, ns])
            fv_chunks.append(fv_st)
        # residual tiles: fvT_sb[p, m, c] = feat_v[c, m*128 + p] + bv[c]
        for co in range(2):
            nc.sync.dma_start_transpose(
                out=fvT_sb[:, :, co * 128 : (co + 1) * 128],
                in_=fvb16[co * 128 : (co + 1) * 128, :],
            )

        # gpsimd queue: the V weights (needed only ~30us in)
        wvT_sb = singles.tile([128, 2, C], BF16, tag="wvT")
        nc.gpsimd.dma_start(out=wvT_sb, in_=wvT)

        # scalar queue: tiny exp to pull the act table in (~2.7us) before
        # the first real exp
        nc.scalar.activation(out=dummy, in_=wu_x[:, 0:8], func=Exp)

        # PE warmup (HAM + covers the first chunk's DMA latency)
        for w in range(10):
            wup = av_psum.tile([128, 512], F32, tag="av")
            nc.tensor.matmul(wup, wu_w, wu_x, start=True, stop=True)

        # ---- building blocks ----
        def proj_chunk(nb):
            ns = bass.ts(nb, IBLK)
            fqkr = qstager.tile([128, 2, IBLK], F32R, tag="fqkr")
            if nb == 0:
                for co in range(2):
                    nc.vector.tensor_copy(
                        out=fqkr[:, co, :], in_=fqk_chunks[nb][:, co, :]
                    )
            else:
                nc.vector.tensor_copy(
                    out=fqkr.rearrange("p a b -> p (a b)"),
                    in_=fqk_chunks[nb].rearrange("p a b -> p (a b)"),
                )
            for (wT_sb, b_sb, dst) in (
                (wqT_sb, bq_sb, q_sb),
                (wkT_sb, bk_sb, k_sb),
            ):
                pp = av_psum.tile([128, 512], F32, tag="av", name="pp")
                for co in range(2):
                    nc.tensor.matmul(
                        pp,
                        wT_sb[:, co, :],
                        fqkr[:, co, :],
                        start=(co == 0),
                        stop=(co == 1),
                    )
                nc.vector.tensor_scalar_add(out=dst[:, ns], in0=pp, scalar1=b_sb)

        def qk_pair(nb, jp, u_list):
            # two j-tiles, concurrent matmuls on alternating PE row sets
            ns = bass.ts(nb, IBLK)
            r = jp % 2
            lp = qk_psum.tile([128, 2 * IBLK], F32, tag="qk")
            for h in range(2):
                jt = 2 * jp + h
                rows = slice(64 * r + 32 * h, 64 * r + 32 * h + 32)
                nc.tensor.matmul(
                    lp[:, h * IBLK : (h + 1) * IBLK],
                    k_sb[rows, bass.ts(jt, JT)],
                    q_sb[rows, ns],
                    start=True,
                    stop=True,
                    tile_position=(64 * r + 32 * h, 0),
                )
            ut = upool.tile([JT, 2 * IBLK], BF16, tag="u")
            nc.scalar.activation(out=ut, in_=lp, func=Exp)
            u_list.append(ut)

        def vproj_pair(k):
            # 2 j-tiles of the V projection in one qk-pool psum tile: the
            # 3-buf lp rotation hides the drain round-trip that serialized
            # the av-pool version at ~0.7us/j-tile
            vp = qk_psum.tile([128, 2, 256], F32, tag="qk", name="vp")
            for u in range(2):
                jt = 2 * k + u
                for co in range(2):
                    nc.tensor.matmul(
                        vp[:, u, :],
                        fv_chunks[jt // 4][:, co, bass.ts(jt % 4, JT)],
                        wvT_sb[:, co, :],
                        start=(co == 0),
                        stop=(co == 1),
                    )
            nc.vector.tensor_copy(
                out=vTx_sb[:, 2 * k : 2 * k + 2, 0:C], in_=vp
            )

        def av_epilogue(avt, itg):
            recip = work.tile([128, 1], F32, tag="recip")
            nc.vector.reciprocal(recip, avt[:, C : C + 1])
            o = opool.tile([128, C], F32, tag="o")
            nc.vector.tensor_scalar(
                out=o, in0=avt[:, 0:C], scalar1=recip, scalar2=None,
                op0=mybir.AluOpType.mult,
            )
            nc.vector.tensor_add(o, o, fvT_sb[:, itg, :])
            nc.sync.dma_start(out=out[bass.ts(itg, 128), :], in_=o)

        # AV work runs as a FIFO of (u_list, itg, chunk) tasks, one 8-matmul
        # chunk per jp slot, lagging the exp stream by one group
        av_tasks = []
        avt_live = {}

        def av_slot():
            u_list, itg, cch = av_tasks.pop(0)
            if cch == 0:
                avt_live[itg] = av_psum.tile(
                    [128, VX + 3], F32, tag="av", name="avt"
                )
            avt = avt_live[itg]
            it = itg % ITPB
            for jt in range(8 * cch, 8 * cch + 8):
                nc.tensor.matmul(
                    avt[:, 0:VX],
                    u_list[jt // 2][
                        :,
                        (jt % 2) * IBLK + it * 128 : (jt % 2) * IBLK + it * 128 + 128,
                    ],
                    vTx_sb[:, jt, 0:VX],
                    start=(jt == 0),
                    stop=(jt == NJT - 1),
                )
            if cch == 3:
                av_epilogue(avt_live.pop(itg), itg)

        # ---- block 0: per-chunk projection + QK/exp ----
        u_cur = []
        for nb in range(NIB):
            proj_chunk(nb)
            qk_pair(0, 2 * nb, u_cur)
            qk_pair(0, 2 * nb + 1, u_cur)

        # ---- blocks 1..7 ----
        prev_u = u_cur
        prev_nb = 0
        for nb in range(1, NIB):
            # it-pair-interleaved order: a group's ch3 (which needs the last
            # vTx tiles) lands as late as possible, and at most two avt
            # groups are ever open (av_psum bufs=2)
            for itp in range(0, ITPB, 2):
                for cch in range(4):
                    for it in (itp, itp + 1):
                        av_tasks.append((prev_u, prev_nb * ITPB + it, cch))
            u_new = []
            # block 1 also runs the V projection, front-loaded; its av
            # slots ramp up only as the vTx tiles they need are emitted
            vp_sched = (3, 3, 3, 3, 2, 2, 0, 0) if nb == 1 else (0,) * 8
            av_sched = (0, 0, 1, 1, 2, 2, 3, 3) if nb == 1 else (2,) * 8
            vpk = 0
            for ci, jp in enumerate(range(0, NJT // 2, 2)):
                for _ in range(vp_sched[ci]):
                    vproj_pair(vpk)
                    vpk += 1
                for _ in range(av_sched[ci]):
                    av_slot()
                qk_pair(nb, jp, u_new)
                qk_pair(nb, jp + 1, u_new)
            prev_u = u_new
            prev_nb = nb

        # tail: remaining AV backlog
        for itp in range(0, ITPB, 2):
            for cch in range(4):
                for it in (itp, itp + 1):
                    av_tasks.append((prev_u, prev_nb * ITPB + it, cch))
        while av_tasks:
            av_slot()


_NC_CACHE = None


def _get_nc():
    global _NC_CACHE
    if _NC_CACHE is None:
        _NC_CACHE = _build_bass()
    return _NC_CACHE


def _pack_qkT(w):
    # [RD, C] -> [128, 2, 4*RD] fp32: wT[p, co, r*RD+d] = w[d, co*128+p]
    wt = np.ascontiguousarray(w, dtype=np.float32).T.reshape(2, 128, RD)
    return np.ascontiguousarray(
        np.tile(wt, (1, 1, 4)).transpose(1, 0, 2)
    )


def _pack_vT(w, bf16):
    # [C, C] -> [128, 2, C] bf16: wvT[p, i, c] = w[c, i*128+p]
    wt = np.ascontiguousarray(w, dtype=np.float32).T.reshape(2, 128, C)
    return np.ascontiguousarray(wt.transpose(1, 0, 2).astype(bf16))


def kernel(x_features, y_features, wqx, bqx, wkx, bkx, wvx, bvx,
           wqy, bqy, wky, bky, wvy, bvy):
    import ml_dtypes

    bf16 = ml_dtypes.bfloat16
    nc = _get_nc()

    def c(a):
        return np.ascontiguousarray(np.asarray(a), dtype=np.float32)

    def rep4(b):
        return np.ascontiguousarray(np.tile(c(b), 4)[:, None])

    in_maps = []
    for b in range(B):
        xf = c(x_features[b]).reshape(C, N)
        yf = c(y_features[b]).reshape(C, N)
        xf16 = np.ascontiguousarray(xf.astype(bf16))
        yf16 = np.ascontiguousarray(yf.astype(bf16))
        xfb16 = np.ascontiguousarray((xf + c(bvx)[:, None]).astype(bf16))
        yfb16 = np.ascontiguousarray((yf + c(bvy)[:, None]).astype(bf16))
        # core 2b: enhanced_x[b] — attention from y features, values from x
        in_maps.append({
            "feat_qk": yf, "fv16": xf16, "fvb16": xfb16,
            "wqT": _pack_qkT(wqy), "wkT": _pack_qkT(wky),
            "wvT": _pack_vT(wvx, bf16),
            "bq4": rep4(bqy), "bk4": rep4(bky),
        })
        # core 2b+1: enhanced_y[b] — attention from x features, values from y
        in_maps.append({
            "feat_qk": xf, "fv16": yf16, "fvb16": yfb16,
            "wqT": _pack_qkT(wqx), "wkT": _pack_qkT(wkx),
            "wvT": _pack_vT(wvy, bf16),
            "bq4": rep4(bqx), "bk4": rep4(bkx),
        })

    res = run_bass_kernel_spmd(nc, in_maps, core_ids=list(range(NCORES)))
    # out_t is [N, C]; flip back to [C, 64, 64]
    outs = [
        np.ascontiguousarray(r["out_t"].T).reshape(C, 64, 64)
        for r in res.results
    ]
    enhanced_x = np.stack(outs[0::2], axis=0)
    enhanced_y = np.stack(outs[1::2], axis=0)
    return enhanced_x, enhanced_y

